# revision 9
# baseline (speedup 1.0000x reference)
"""Trainium2 Bass kernel for the DCM sparse-attention problem.

Math restructure: with t-hat/v-hat the row-normalized features and
S[(a,t),(b,v)] = <t-hat[a,t], v-hat[b,v]> the raw cosine logits, every
softmax-weighted aggregation in the reference collapses onto S:

  t2v[a,b,t] = sum_v vps1 * S          (free-dim group reduce)
  v2t[a,b,v] = sum_t tps1 * S          (partition reduce via indicator matmul)
  out[a,b]   = sum_t sum_v tps2[t] * vps2[v] * S[t,v]

so the [A,B,T,D] intermediates never exist. Each of the 8 cores handles
8 of the 64 text rows (A-sharding, video replicated); no collectives.
"""

import sys

sys.path.insert(0, "/opt/trn_rl_repo")

import numpy as np

import concourse.bass as bass
import concourse.bacc as bacc
import concourse.tile as tile
from concourse import mybir
from concourse.bass_utils import run_bass_kernel_spmd

TAU = 100.0
EPS = 1e-6
A, T, B, V, D = 64, 32, 64, 12, 512
NCORES = 8
AL = A // NCORES          # a's per core = 8
AT = AL * T               # (a,t) rows per core = 256
BV = B * V                # (b,v) cols = 768
NMT = AT // 128           # M-tiles over (a,t) = 2
NKT = D // 128            # K-tiles over d = 4
APB = 128 // T            # a's per M-tile = 4
F32 = mybir.dt.float32
NSL = [(0, 512), (512, 768)]            # bank-aligned N-slices of 768
NSL2 = [(0, 512), (512, 1024), (1024, 1536)]  # ... of 1536


def _build_program():
    nc = bacc.Bacc("TRN2", target_bir_lowering=False)

    tT_d = nc.declare_dram_parameter("tT", [D, AT], F32, isOutput=False)
    vT_d = nc.declare_dram_parameter("vT", [D, BV], F32, isOutput=False)
    mask_d = nc.declare_dram_parameter("mask", [AT, 1], F32, isOutput=False)
    ident_d = nc.declare_dram_parameter("ident", [128, 128], F32, isOutput=False)
    ind4_d = nc.declare_dram_parameter("ind4", [128, APB], F32, isOutput=False)
    ind8_d = nc.declare_dram_parameter("ind8", [AL, AT], F32, isOutput=False)
    onesc_d = nc.declare_dram_parameter("onesc", [128, 1], F32, isOutput=False)
    onesr_d = nc.declare_dram_parameter("onesr", [1, 128], F32, isOutput=False)
    ind4T_d = nc.declare_dram_parameter("ind4T", [APB, 128], F32, isOutput=False)
    out_d = nc.declare_dram_parameter("out", [AL, B], F32, isOutput=True)

    with tile.TileContext(nc) as tc:
        with (
            tc.tile_pool(name="consts", bufs=1) as consts,
            tc.tile_pool(name="inputs", bufs=1) as inputs,
            tc.tile_pool(name="sq", bufs=3) as sqp,
            tc.tile_pool(name="big", bufs=1) as bigp,
            tc.tile_pool(name="smalls", bufs=1) as smalls,
            tc.tile_pool(name="psum_big", bufs=2, space="PSUM") as ppool,
            tc.tile_pool(name="psum_small", bufs=2, space="PSUM") as psmall,
        ):
            # ---- constants / small inputs ----
            ident = consts.tile([128, 128], F32)
            nc.sync.dma_start(out=ident, in_=ident_d[:, :])
            ind4 = consts.tile([128, APB], F32)
            nc.sync.dma_start(out=ind4, in_=ind4_d[:, :])
            ind8 = consts.tile([AL, AT], F32)
            nc.sync.dma_start(out=ind8, in_=ind8_d[:, :])
            onesc = consts.tile([128, 1], F32)
            nc.sync.dma_start(out=onesc, in_=onesc_d[:, :])
            onesr = consts.tile([1, 128], F32)
            nc.sync.dma_start(out=onesr, in_=onesr_d[:, :])
            ind4T = consts.tile([APB, 128], F32)
            nc.sync.dma_start(out=ind4T, in_=ind4T_d[:, :])
            maskt = [consts.tile([128, 1], F32, tag=f"m{i}", name=f"maskt{i}") for i in range(NMT)]
            for i in range(NMT):
                nc.sync.dma_start(out=maskt[i], in_=mask_d[128 * i:128 * (i + 1), :])
            tau_m = [consts.tile([128, 1], F32, tag=f"tm{i}", name=f"tau_m{i}") for i in range(NMT)]
            for i in range(NMT):
                nc.vector.tensor_scalar_mul(tau_m[i], maskt[i], TAU)

            # ---- main inputs ([d, row] layouts) ----
            tT = [inputs.tile([128, AT], F32, tag=f"tT{k}", name=f"tT{k}") for k in range(NKT)]
            vT = [inputs.tile([128, BV], F32, tag=f"vT{k}", name=f"vT{k}") for k in range(NKT)]
            for k in range(NKT):
                nc.sync.dma_start(out=tT[k], in_=tT_d[128 * k:128 * (k + 1), :])
                nc.sync.dma_start(out=vT[k], in_=vT_d[128 * k:128 * (k + 1), :])

            # ---- norms: column sums of squares via ones-matmul ----
            ps_sst = psmall.tile([1, AT], F32, tag="small")
            ps_ssv = ppool.tile([1, BV], F32, tag="big")
            for k in range(NKT):
                sq = sqp.tile([128, BV], F32, tag="sq")
                nc.scalar.square(sq[:, :AT], tT[k])
                nc.tensor.matmul(ps_sst, onesc, sq[:, :AT],
                                 start=(k == 0), stop=(k == NKT - 1))
                sq2 = sqp.tile([128, BV], F32, tag="sq")
                nc.scalar.square(sq2, vT[k])
                for lo, hi in NSL:
                    nc.tensor.matmul(ps_ssv[:, lo:hi], onesc, sq2[:, lo:hi],
                                     start=(k == 0), stop=(k == NKT - 1))

            # r = 1/max(sqrt(ss), eps), in row layout
            nt_row = smalls.tile([1, AT], F32)
            nc.scalar.sqrt(nt_row, ps_sst)
            nc.vector.tensor_scalar_max(nt_row, nt_row, EPS)
            rv_row = smalls.tile([1, BV], F32)
            nc.scalar.sqrt(rv_row, ps_ssv)
            nc.vector.tensor_scalar_max(rv_row, rv_row, EPS)
            nc.vector.reciprocal(rv_row, rv_row)

            # transpose text norms to a [at, 1] column; reciprocal on the way out
            r_t = [smalls.tile([128, 1], F32, tag=f"rt{i}", name=f"r_t{i}") for i in range(NMT)]
            for i in range(NMT):
                ps_tr = psmall.tile([128, 1], F32, tag="small")
                nc.tensor.transpose(ps_tr, nt_row[:, 128 * i:128 * (i + 1)],
                                    ident[0:1, 0:1])
                nc.vector.reciprocal(r_t[i], ps_tr)

            # broadcast rv across partitions (ones outer product), scale vT cols
            ps_rv = ppool.tile([128, BV], F32, tag="big")
            for lo, hi in NSL:
                nc.tensor.matmul(ps_rv[:, lo:hi], onesr, rv_row[:, lo:hi],
                                 start=True, stop=True)
            for k in range(NKT):
                nc.vector.tensor_mul(vT[k], vT[k], ps_rv)

            # ---- S = t-hat @ v-hat.T, tiled [128, 768] per M-tile ----
            S = [bigp.tile([128, BV], F32, tag=f"S{i}", name=f"S{i}") for i in range(NMT)]
            E = [bigp.tile([128, BV], F32, tag=f"E{i}", name=f"E{i}") for i in range(NMT)]
            rhs_v = [bigp.tile([128, 2 * BV], F32, tag=f"rv{i}", name=f"rhs_v{i}") for i in range(NMT)]
            rhs_f = [smalls.tile([128, 128], F32, tag=f"rf{i}", name=f"rhs_f{i}") for i in range(NMT)]
            v2t_sb = [smalls.tile([APB, BV], F32, tag=f"v2t_sb{i}", name=f"v2t_sb{i}") for i in range(NMT)]
            E4 = [smalls.tile([APB, BV], F32, tag=f"E4_{i}", name=f"E4_{i}") for i in range(NMT)]
            rD4 = [smalls.tile([APB, B], F32, tag=f"rD4_{i}", name=f"rD4_{i}") for i in range(NMT)]

            for i in range(NMT):
                ps_s = ppool.tile([128, BV], F32, tag="big")
                for lo, hi in NSL:
                    for k in range(NKT):
                        nc.tensor.matmul(
                            ps_s[:, lo:hi],
                            tT[k][:, 128 * i:128 * (i + 1)],
                            vT[k][:, lo:hi],
                            start=(k == 0), stop=(k == NKT - 1))
                # S = psum * r_t (row scale), E = exp(tau*m*S)
                nc.vector.tensor_scalar_mul(S[i], ps_s, r_t[i])
                nc.scalar.activation(E[i], S[i], mybir.ActivationFunctionType.Exp,
                                     scale=tau_m[i][:, :])
                # rhs_v = [E2S | E2] with E2 = m*E
                nc.vector.tensor_scalar_mul(rhs_v[i][:, BV:], E[i], maskt[i])
                nc.vector.tensor_mul(rhs_v[i][:, :BV], rhs_v[i][:, BV:], S[i])
                # t2v = group-sum(E*S) / group-sum(E) over v
                es = sqp.tile([128, BV], F32, tag="sq")
                nc.vector.tensor_mul(es, E[i], S[i])
                nm = smalls.tile([128, B], F32, tag=f"nm{i}")
                dn = smalls.tile([128, B], F32, tag=f"dn{i}")
                nc.vector.reduce_sum(nm, es.rearrange("p (b v) -> p b v", v=V),
                                     axis=mybir.AxisListType.X)
                nc.vector.reduce_sum(dn, E[i].rearrange("p (b v) -> p b v", v=V),
                                     axis=mybir.AxisListType.X)
                nc.vector.reciprocal(dn, dn)
                t2v = smalls.tile([128, B], F32, tag=f"t2v{i}")
                nc.vector.tensor_mul(t2v, nm, dn)
                # E3 = exp(tau * t2v) into rhs_f[:, 64:]
                nc.scalar.activation(rhs_f[i][:, B:], t2v,
                                     mybir.ActivationFunctionType.Exp, scale=TAU)
                # v2t numerator/denominator: indicator matmul over t
                ps_v = ppool.tile([APB, 2 * BV], F32, tag="big")
                for lo, hi in NSL2:
                    nc.tensor.matmul(ps_v[:, lo:hi], ind4, rhs_v[i][:, lo:hi],
                                     start=True, stop=True)
                rd = smalls.tile([APB, BV], F32, tag=f"rd{i}")
                nc.vector.reciprocal(rd, ps_v[:, BV:])
                nc.vector.tensor_mul(v2t_sb[i], ps_v[:, :BV], rd)

            # ---- vps2 path (per M-tile of 4 a's) ----
            for i in range(NMT):
                nc.scalar.activation(E4[i], v2t_sb[i],
                                     mybir.ActivationFunctionType.Exp, scale=TAU)
                D4 = smalls.tile([APB, B], F32, tag=f"D4_{i}", name=f"D4_{i}")
                nc.vector.reduce_sum(D4, E4[i].rearrange("p (b v) -> p b v", v=V),
                                     axis=mybir.AxisListType.X)
                nc.vector.reciprocal(rD4[i], D4)

            for i in range(NMT):
                # W4 = E4 rows broadcast to their 32 t-rows (indicator matmul)
                ps_w = ppool.tile([128, BV], F32, tag="big")
                for lo, hi in NSL:
                    nc.tensor.matmul(ps_w[:, lo:hi], ind4T, E4[i][:, lo:hi],
                                     start=True, stop=True)
                w4s = sqp.tile([128, BV], F32, tag="sq")
                nc.vector.tensor_mul(w4s, ps_w, S[i])
                hun = smalls.tile([128, B], F32, tag=f"hun{i}")
                nc.vector.reduce_sum(hun, w4s.rearrange("p (b v) -> p b v", v=V),
                                     axis=mybir.AxisListType.X)
                nc.vector.tensor_mul(rhs_f[i][:, :B], rhs_f[i][:, B:], hun)
                # final indicator matmul: rows a, cols [out_un | D3]
                ps_o = psmall.tile([APB, 128], F32, tag="small")
                nc.tensor.matmul(ps_o, ind4, rhs_f[i], start=True, stop=True)
                r3 = smalls.tile([APB, B], F32, tag=f"r3{i}")
                nc.vector.reciprocal(r3, ps_o[:, B:])
                nc.vector.tensor_mul(r3, r3, rD4[i])
                out_sb = smalls.tile([APB, B], F32, tag=f"osb{i}", name=f"out_sb{i}")
                nc.vector.tensor_mul(out_sb, ps_o[:, :B], r3)
                nc.sync.dma_start(out=out_d[APB * i:APB * (i + 1), :], in_=out_sb)

    nc.compile()
    return nc


_NC_CACHE = None


def _get_program():
    global _NC_CACHE
    if _NC_CACHE is None:
        _NC_CACHE = _build_program()
    return _NC_CACHE


def _make_in_maps(text_feat, video_feat, text_mask):
    vT = np.ascontiguousarray(video_feat.reshape(BV, D).T)
    ident = np.eye(128, dtype=np.float32)
    ind4 = np.zeros((128, APB), np.float32)
    ind4[np.arange(128), np.arange(128) // T] = 1.0
    ind8 = np.zeros((AL, AT), np.float32)
    ind8[np.arange(AT) // T, np.arange(AT)] = 1.0
    onesc = np.ones((128, 1), np.float32)
    onesr = np.ones((1, 128), np.float32)
    in_maps = []
    for c in range(NCORES):
        tsl = text_feat[c * AL:(c + 1) * AL].reshape(AT, D)
        in_maps.append({
            "tT": np.ascontiguousarray(tsl.T),
            "vT": vT,
            "mask": text_mask[c * AL:(c + 1) * AL].reshape(AT, 1).astype(np.float32),
            "ident": ident,
            "ind4": ind4,
            "ind4T": np.ascontiguousarray(ind4.T),
            "ind8": ind8,
            "onesc": onesc,
            "onesr": onesr,
        })
    return in_maps


def kernel(text_feat, video_feat, text_mask, _trace=False):
    text_feat = np.asarray(text_feat, dtype=np.float32)
    video_feat = np.asarray(video_feat, dtype=np.float32)
    text_mask = np.asarray(text_mask)
    nc = _get_program()
    in_maps = _make_in_maps(text_feat, video_feat, text_mask)
    res = run_bass_kernel_spmd(nc, in_maps, core_ids=list(range(NCORES)),
                               trace=_trace)
    out = np.concatenate([res.results[c]["out"] for c in range(NCORES)], axis=0)
    if _trace:
        kernel.last_exec_time_ns = res.exec_time_ns
        kernel.last_results = res
    return out
